# revision 5
# baseline (speedup 1.0000x reference)
"""Trainium2 Bass kernel for a margin-softmax cross-entropy loss.

Reference computation (B=4096, D=512, C=10575):
    original = feats @ w                         # [B, C]
    means    = centers / counts[:, None]
    mn       = means / ||means||                 # unit rows
    dists    = mn[labels] @ mn.T                 # [B, C]
    logits   = original + onehot(labels) * dists # only label column changes
    loss     = mean(CE(logits, labels))

Key identities used (see kernel_baseline.py for the fuller derivation):
  * only the label column of `dists` survives the onehot mask and equals
    |mn|^2 ~ 1.0 (host-computed);
  * CE needs only S_i = sum_j exp(l_ij) and t_i = l_i,label(i).

Device work per core (classes sharded 8 ways, 1328 padded/core): one
[4096 x 512] @ [512 x 1328] fp8 DoubleRow GEMM; per 128-row batch strip
the row-sums of exp(logits) are computed by one of two paths so that the
Scalar (ACT) engine is no longer the single bottleneck:

  * ACT strips: nc.scalar.activation(Exp) PSUM->SBUF fp16 (1 elem/cyc
    @1.2GHz), then the row-sum runs as a DVE tensor_scalar with
    accum_out on the fp16 tile (4x mode: 0.25 cyc/elem) or as a GpSimd
    tensor_scalar accum (SBUF only, ~0.6 efficiency).
  * DVE strips: Schraudolph exp2 bit-trick - one 1x DVE tensor_scalar
    reads the PSUM fp32 logits and writes round(l*A + B) as int16;
    reinterpreted as bf16 that IS ~exp(l/WSCALE) (linear-in-mantissa
    exp2 approx, +4.04%% bias calibrated into B and corrected on host);
    a second 4x tensor_scalar sums the bitcast bf16 with accum_out.

The label logits t_i come from a tiny "diagonal GEMM" (w[:, labels]
gathered on host); the host combines the 8 cores' partial sums, fixes
the pad columns and the D-strip bias, and applies the margin:
    S' = S - exp(t) + exp(t + d);  nll = log(S') - (t + d).
"""

from contextlib import ExitStack

import ml_dtypes
import numpy as np

import concourse.bass as bass
import concourse.tile as tile
from concourse import bacc, mybir
from concourse.bass_utils import run_bass_kernel_spmd

B = 4096
D = 512
C = 10575
NCORES = 8
CS_BASE = 1322        # real classes on cores 0..6; core 7 gets 1321
CSH = 1328            # padded per-core class count
CW = (512, 512, 304)  # class-tile widths (PSUM bank-aligned starts)
CO = (0, 512, 1024)
KT = D // 128
KP = 2                # fp8 DoubleRow k-pairs (256 contraction each)
BT = B // 128         # 32 batch tiles
BSH = B // NCORES     # 512 rows of label logits per core
JT = BSH // 128       # 4 diagonal sub-tiles
WSCALE = 64.0         # fp8 pre-scale for w (subnormal-range fix)

BF16 = mybir.dt.bfloat16
FP16 = mybir.dt.float16
FP8 = mybir.dt.float8e4
F32 = mybir.dt.float32
I16 = mybir.dt.int16

# ---- strip scheduling -------------------------------------------------
# D-strips use the DVE bit-trick path; the rest are ACT(exp) strips.
# The last ACT strips fold their row-sum into the activation's accum_out
# (+187ns on ACT but no DVE dependency -> clean pipeline tail); earlier
# ACT strips sum on DVE via a 4x-mode tensor_scalar accum.
D_STRIPS = frozenset((2, 4, 7, 10, 12, 15, 18, 20, 23, 26, 29))
_A_STRIPS = tuple(b for b in range(32) if b not in D_STRIPS)
ACT_ACC_STRIPS = frozenset(_A_STRIPS[-9:])

# Schraudolph constants: i16 = rint(l_scaled * A_TRICK + B_TRICK);
# bitcast bf16 == exp(l_scaled / WSCALE) * (1 + bias), bias ~ +4.04e-2
# folded into B_TRICK below; TRICK_BIAS holds the residual measured on
# hardware (host divides D-path sums by 1 + TRICK_BIAS).
LN2 = 0.6931471805599453
TRICK_RAW_BIAS = 4.04e-2
A_TRICK = 128.0 / (LN2 * WSCALE)
B_TRICK = 16256.0 - 128.0 * np.log2(1.0 + TRICK_RAW_BIAS)
TRICK_BIAS = 0.0
# pad columns (logit 0) contribute bitcast(rint(B_TRICK)) on D-strips
PAD_VAL_D = float(
    np.int16(np.rint(B_TRICK)).view(ml_dtypes.bfloat16).astype(np.float64)
)

_CACHE = {}


def _build_nc():
    nc = bacc.Bacc("TRN2", debug=False, target_bir_lowering=False)

    # Split the input pack so strip 0 can start as early as possible:
    #   headA: wS c-tile0 (both k) + fT0 (both k)      -> first matmuls
    #   headB: wS c-tiles 1+2 (both k)                 -> rest of strip 0
    #   headC: wL (diag) + identity + nothing else     -> diag GEMM at b==2
    #   fTr:   remaining feat columns (b-tiles 4..31)
    AW = KP * 2 * CW[0] + KP * 2 * 512
    BW = KP * 2 * (CW[1] + CW[2])
    CWID = KP * 2 * BSH + 2 * JT * 128
    headA = nc.dram_tensor("headA", [128, AW], FP8, kind="ExternalInput").ap()
    headB = nc.dram_tensor("headB", [128, BW], FP8, kind="ExternalInput").ap()
    headC = nc.dram_tensor("headC", [128, CWID], FP8, kind="ExternalInput").ap()
    fTr = nc.dram_tensor("fTr", [KP, 128, 2, B - 512], FP8, kind="ExternalInput").ap()
    outS = nc.dram_tensor("outS", [128, BT], F32, kind="ExternalOutput").ap()
    outT = nc.dram_tensor("outT", [128, JT], F32, kind="ExternalOutput").ap()

    with tile.TileContext(nc) as tc, ExitStack() as ctx:
        consts = ctx.enter_context(tc.tile_pool(name="consts", bufs=1))
        psums = ctx.enter_context(tc.tile_pool(name="psums", bufs=2, space="PSUM"))
        psumd = ctx.enter_context(tc.tile_pool(name="psumd", bufs=1, space="PSUM"))
        psumw = ctx.enter_context(tc.tile_pool(name="psumw", bufs=1, space="PSUM"))
        epool = ctx.enter_context(tc.tile_pool(name="epool", bufs=8))
        ipool = ctx.enter_context(tc.tile_pool(name="ipool", bufs=3))
        spool = ctx.enter_context(tc.tile_pool(name="spool", bufs=2))
        work = ctx.enter_context(tc.tile_pool(name="work", bufs=2))
        outs = ctx.enter_context(tc.tile_pool(name="outs", bufs=1))

        # ACT table prefetch: a dummy exp on a zeroed [128,1] SBUF tile at
        # t=0 pulls the ~1.3us exp table load off the critical path.
        dumm = consts.tile([128, 1], F32, tag="dumm")
        nc.vector.memset(dumm[:], 0.0)
        de = consts.tile([128, 1], FP16, tag="de")
        nc.scalar.activation(out=de[:], in_=dumm[:],
                             func=mybir.ActivationFunctionType.Exp)

        # PE HAM warm-up during the DMA head: real matmuls start at 2.4GHz
        warm = consts.tile([128, 512], BF16, tag="warm")
        nc.vector.memset(warm[:], 0.0)
        pw = psumw.tile([128, 512], F32, tag="pw")
        for _ in range(6):
            nc.tensor.matmul(out=pw[:], lhsT=warm[:, 0:128], rhs=warm[:],
                             start=True, stop=True)

        headA_sb = consts.tile([128, AW], FP8, tag="headA")
        headB_sb = consts.tile([128, BW], FP8, tag="headB")
        headC_sb = consts.tile([128, CWID], FP8, tag="headC")
        dA = nc.sync.dma_start(out=headA_sb[:], in_=headA[:])
        dB = nc.sync.dma_start(out=headB_sb[:], in_=headB[:])
        tile.add_dep_helper(dB.ins, dA.ins, reason="headB after headA")
        dC = nc.sync.dma_start(out=headC_sb[:], in_=headC[:])
        tile.add_dep_helper(dC.ins, dB.ins, reason="headC after headB")

        # SBUF views into the packs
        wS_c0 = [
            headA_sb[:, k * 2 * CW[0]:(k + 1) * 2 * CW[0]].rearrange(
                "p (i n) -> p i n", i=2)
            for k in range(KP)
        ]
        FOFF = KP * 2 * CW[0]
        fT0_sb = [
            headA_sb[:, FOFF + k * 1024:FOFF + (k + 1) * 1024].rearrange(
                "p (i n) -> p i n", i=2)
            for k in range(KP)
        ]
        W12 = CW[1] + CW[2]
        wS_c12 = [
            headB_sb[:, k * 2 * W12:(k + 1) * 2 * W12].rearrange(
                "p (i n) -> p i n", i=2)
            for k in range(KP)
        ]
        wL_sb = [
            headC_sb[:, k * 2 * BSH:(k + 1) * 2 * BSH].rearrange(
                "p (i n) -> p i n", i=2)
            for k in range(KP)
        ]
        IOFF = KP * 2 * BSH
        id_sb = headC_sb[:, IOFF:IOFF + 2 * JT * 128].bitcast(BF16)

        def rhs_for(k, c):
            if c == 0:
                return wS_c0[k]
            if c == 1:
                return wS_c12[k][:, :, 0:CW[1]]
            return wS_c12[k][:, :, CW[1]:W12]

        # remaining fT columns, gated behind the critical pack
        fTr_sb = []
        for k in range(KP):
            t = consts.tile([128, 2, B - 512], FP8, tag=f"fTr{k}")
            fTr_sb.append(t)
        a_dmas = []
        for k in range(KP):
            d = nc.sync.dma_start(
                out=fTr_sb[k][:, :, 0:1536], in_=fTr[k][:, :, 0:1536])
            tile.add_dep_helper(d.ins, dC.ins, reason="fTr-a waits for heads")
            a_dmas.append(d)
        for k in range(KP):
            d = nc.sync.dma_start(
                out=fTr_sb[k][:, :, 1536:B - 512], in_=fTr[k][:, :, 1536:B - 512])
            for ad in a_dmas:
                tile.add_dep_helper(d.ins, ad.ins, reason="fTr-b waits for fTr-a")

        st = outs.tile([128, BT], F32, tag="st")
        for b in range(BT):
            ps = psums.tile([128, CSH], F32, tag="ps")
            for k in range(KP):
                for c in range(len(CW)):
                    lhsT = (fT0_sb[k][:, :, b * 128:(b + 1) * 128]
                            if b < 4 else
                            fTr_sb[k][:, :, (b - 4) * 128:(b - 3) * 128])
                    nc.tensor.matmul(
                        out=ps[:, CO[c]:CO[c] + CW[c]],
                        lhsT=lhsT,
                        rhs=rhs_for(k, c),
                        start=(k == 0),
                        stop=(k == KP - 1),
                        perf_mode=mybir.MatmulPerfMode.DoubleRow,
                    )

            if b in D_STRIPS:
                # bit-trick path: i16 = rint(ps * A + B); bf16-bitcast is
                # ~exp(ps/WSCALE); 4x tensor_scalar sums it into st.
                it = ipool.tile([128, CSH], I16, tag="it")
                nc.vector.tensor_scalar(
                    out=it[:], in0=ps[:],
                    scalar1=float(A_TRICK), scalar2=float(B_TRICK),
                    op0=mybir.AluOpType.mult, op1=mybir.AluOpType.add,
                )
                sc = spool.tile([128, CSH], BF16, tag="sc")
                nc.vector.tensor_scalar(
                    out=sc[:], in0=it[:].bitcast(BF16),
                    scalar1=1.0, scalar2=0.0,
                    op0=mybir.AluOpType.mult, op1=mybir.AluOpType.add,
                    accum_out=st[:, b:b + 1],
                )
            else:
                e = epool.tile([128, CSH], FP16, tag="e")
                if b in ACT_ACC_STRIPS:
                    nc.scalar.activation(
                        out=e[:], in_=ps[:],
                        func=mybir.ActivationFunctionType.Exp,
                        scale=float(1.0 / WSCALE),
                        accum_out=st[:, b:b + 1],
                    )
                else:
                    nc.scalar.activation(
                        out=e[:], in_=ps[:],
                        func=mybir.ActivationFunctionType.Exp,
                        scale=float(1.0 / WSCALE),
                    )
                    sc = spool.tile([128, CSH], FP16, tag="scv")
                    nc.vector.tensor_scalar(
                        out=sc[:], in0=e[:], scalar1=1.0, scalar2=0.0,
                        op0=mybir.AluOpType.mult, op1=mybir.AluOpType.add,
                        accum_out=st[:, b:b + 1],
                    )

            if b == 2:
                # diag(fSel.T @ wL): label logits (x WSCALE^2)
                tt = outs.tile([128, JT], F32, tag="tt")
                pd = psumd.tile([128, JT * 128], F32, tag="pd")
                for j in range(JT):
                    for k in range(KP):
                        nc.tensor.matmul(
                            out=pd[:, j * 128:(j + 1) * 128],
                            lhsT=fT0_sb[k][:, :, j * 128:(j + 1) * 128],
                            rhs=wL_sb[k][:, :, j * 128:(j + 1) * 128],
                            start=(k == 0),
                            stop=(k == KP - 1),
                            perf_mode=mybir.MatmulPerfMode.DoubleRow,
                        )
                scr = work.tile([128, JT * 128], F32, tag="scr")
                nc.vector.tensor_mul(out=scr[:], in0=id_sb[:], in1=pd[:])
                for j in range(JT):
                    nc.vector.tensor_reduce(
                        out=tt[:, j:j + 1], in_=scr[:, j * 128:(j + 1) * 128],
                        axis=mybir.AxisListType.X, op=mybir.AluOpType.add,
                    )
                nc.sync.dma_start(out=outT[:], in_=tt[:])
            if b == BT // 2 - 1:
                nc.sync.dma_start(out=outS[:, 0:BT // 2], in_=st[:, 0:BT // 2])
        nc.sync.dma_start(out=outS[:, BT // 2:], in_=st[:, BT // 2:])

    nc.compile()
    return nc


def _core_sizes():
    sizes = [CS_BASE] * (NCORES - 1) + [C - CS_BASE * (NCORES - 1)]
    starts = np.concatenate([[0], np.cumsum(sizes)[:-1]]).astype(np.int64)
    return np.array(sizes, dtype=np.int64), starts


def _prepare_inputs(feats, labels, w):
    sizes, starts = _core_sizes()
    ident = np.ascontiguousarray(
        np.tile(np.eye(128, dtype=np.float32), (1, JT))
    ).astype(ml_dtypes.bfloat16)
    ident_bytes = np.ascontiguousarray(ident).view(np.uint8).reshape(128, -1)

    in_maps = []
    for p in range(NCORES):
        frolled = np.roll(feats, -p * BSH, axis=0)
        fT_host = np.ascontiguousarray(
            frolled.reshape(B, KP, 2, 128).transpose(1, 3, 2, 0)
        ).astype(ml_dtypes.float8_e4m3)
        fTr_host = np.ascontiguousarray(fT_host[:, :, :, 512:])
        c0, sz = int(starts[p]), int(sizes[p])
        wp = np.zeros((D, CSH), dtype=np.float32)
        wp[:, :sz] = w[:, c0:c0 + sz] * WSCALE
        wS_host = np.ascontiguousarray(
            wp.reshape(KP, 2, 128, CSH).transpose(0, 2, 1, 3)
        ).astype(ml_dtypes.float8_e4m3)

        rows = slice(p * BSH, (p + 1) * BSH)
        wlab = (w[:, labels[rows]] * WSCALE)                      # [D, BSH]
        wL_host = np.ascontiguousarray(
            wlab.reshape(KP, 2, 128, BSH).transpose(0, 2, 1, 3)
        ).astype(ml_dtypes.float8_e4m3)

        headA_host = np.concatenate(
            [wS_host[k][:, :, 0:CW[0]].reshape(128, -1) for k in range(KP)]
            + [np.ascontiguousarray(fT_host[k][:, :, 0:512]).reshape(128, -1)
               for k in range(KP)],
            axis=1,
        )
        headB_host = np.concatenate(
            [np.ascontiguousarray(wS_host[k][:, :, CW[0]:CSH]).reshape(128, -1)
             for k in range(KP)],
            axis=1,
        )
        headC_host = np.concatenate(
            [wL_host[k].reshape(128, -1) for k in range(KP)]
            + [ident_bytes.view(ml_dtypes.float8_e4m3)],
            axis=1,
        )
        in_maps.append({
            "headA": np.ascontiguousarray(headA_host),
            "headB": np.ascontiguousarray(headB_host),
            "headC": np.ascontiguousarray(headC_host),
            "fTr": fTr_host,
        })
    return in_maps


def _run(in_maps, trace=False):
    if "nc" not in _CACHE:
        _CACHE["nc"] = _build_nc()
    nc = _CACHE["nc"]
    return run_bass_kernel_spmd(
        nc, in_maps, core_ids=list(range(NCORES)), trace=trace
    )


def kernel(feats, labels, centers, counts, w, _trace=False, _ret_res=False):
    feats = np.asarray(feats, dtype=np.float32)
    labels_i = np.asarray(labels).astype(np.int64)
    centers = np.asarray(centers, dtype=np.float32)
    counts = np.asarray(counts, dtype=np.float32)
    w = np.asarray(w, dtype=np.float32)

    in_maps = _prepare_inputs(feats, labels_i, w)
    res = _run(in_maps, trace=_trace)

    sizes, starts = _core_sizes()

    # margin d_c = |means_c / ||means_c|| |^2 (~1.0)
    means = (centers / counts[:, None]).astype(np.float32)
    nrm = np.sqrt((means.astype(np.float32) ** 2).sum(axis=1, keepdims=True))
    mn = (means / nrm).astype(np.float32)
    dsq = (mn.astype(np.float64) ** 2).sum(axis=1)       # [C]
    d = dsq[labels_i]                                    # [B]

    d_cols = np.array([b in D_STRIPS for b in range(BT)])
    S_tot = np.zeros(B, dtype=np.float64)
    t = np.empty(B, dtype=np.float64)
    for p in range(NCORES):
        S_p = res.results[p]["outS"].astype(np.float64)  # [128, BT]
        pads = float(CSH - sizes[p])
        S_p[:, d_cols] = (S_p[:, d_cols] - pads * PAD_VAL_D) / (1.0 + TRICK_BIAS)
        S_p[:, ~d_cols] -= pads
        S_p = S_p.T.reshape(B)
        S_tot += np.roll(S_p, p * BSH)
        T_p = res.results[p]["outT"].astype(np.float64)  # [128, JT]
        t[p * BSH:(p + 1) * BSH] = T_p.T.reshape(BSH) / WSCALE

    z = S_tot - np.exp(t) + np.exp(t + d)
    nll = np.log(z) - (t + d)
    loss = np.float32(nll.mean())
    out = np.array(loss, dtype=np.float32)
    if _ret_res:
        return out, res
    return out
